# revision 48
# baseline (speedup 1.0000x reference)
"""Trainium2 Bass kernel for the dynamic-filter CNN (DCM) module.

Reference computation (per sample b):
  pooled    = adaptive_avg_pool2d(x[b], (3,3))                  # [Cin,3,3]
  gen_filt  = filter_gen_w @ pooled + filter_gen_b              # [C,3,3]
  xr        = relu(redu_w @ x[b] + redu_b)                      # [C,H,W]
  dw        = relu(depthwise3x3(xr, gen_filt, zero-pad 1))      # [C,H,W]
  out       = relu(fusion_w @ dw + fusion_b)                    # [C,H,W]

Sharding: 8 cores = (batch 4) x (H-half 2). Each core owns 32 output rows and
loads one halo row each side. Bottom-half cores receive their rows REVERSED by
the host so a single SPMD graph works for all cores; the 3x3 filter is
mirrored per-core and the adaptive-pool bin placement resolved per-core via
tiny host-supplied 0/1 mask tensors + a pair-wise AllReduce.

Shapes hardcoded for x=[4,2048,64,64] f32, C=512.

v4 schedule, driven by trace findings:
- pool partials are ONE strided tensor_reduce per arriving x tile (w-bins
  with the overlapping-window AP trick), split DVE/GpSimd to keep pace with
  the stream; row-binning is two batched reduces after the last tile. This
  replaces the serialized ACT accumulator chain that used to lag the stream
  by ~10us.
- the scatter/dump/AllReduce trigger chain runs on GpSimd (idle, in-order)
  so the scheduler cannot push it behind eviction work; the dump rides the
  sync ring right after the x stream; payload is bf16 (36KB) to cut
  collective time; a dummy warm-up AllReduce at t=0 absorbs the ncfw
  start-up latency.
- redu conv halo rows + the filter-gen matmul are deferred to AFTER pass B
  so PE has work while the collective completes.
- diag tiles for the PE depthwise are built on ACT (activation scale=tap),
  freeing DVE to run four depthwise row-block units via scalar_tensor_tensor
  (the bi=3 quarter), trimming the PE-serial depthwise from 31 to ~23us.
- depthwise + fusion interleave by row-block pair; output streams per
  (co, block) chunk on the sync ring, with tiny SBUF->DRAM "pre-wake" DMAs
  keyed to mid-kernel tiles so the ring's ~9us idle-wakeup latency is paid
  before the first real output chunk.
"""
import os
import numpy as np
import ml_dtypes

import concourse.bass as bass
import concourse.mybir as mybir
import concourse.tile as tile
from concourse.bass_utils import run_bass_kernel_spmd
from concourse.vector_clock import ScopedClock

F32 = mybir.dt.float32
BF16 = mybir.dt.bfloat16


# Workaround for this container's walrus codegen: an instruction's inline sync
# header only supports one wait command ("Too many sync wait commands" in
# CoreV3GenImpl setupSyncWait), but Tile's kernel-tail drain attaches one wait
# per logical proc. Spread the drain's waits across preceding nofuse NOPs on
# the same engine (program order keeps the drain after all of them).
def _patched_drain_and_barrier(self, tick_clock, wait_clock):
    nops = [self.nc.sync.nop(nofuse=True, hint="drain_wait_spread")
            for _ in range(28)]
    drain_inst = self.nc.sync.drain()
    wait_clock.add_sem_waits(
        drain_inst.ins, ScopedClock({None: tick_clock.global_clock}))
    si = drain_inst.ins.sync_info
    waits = list(si.on_wait) if si is not None and si.on_wait else []
    if len(waits) > 1:
        assert len(waits) <= len(nops) + 1, f"too many drain waits: {len(waits)}"
        for i, wentry in enumerate(waits[1:]):
            nops[i].ins.sync_info = mybir.SyncInfo(
                on_wait=[wentry], on_update=[])
        drain_inst.ins.sync_info = mybir.SyncInfo(
            on_wait=[waits[0]], on_update=list(si.on_update or []))
    self.nc.all_engine_barrier()
    popped = self.nc._tile_sem_poison_stack.pop()
    assert popped is self._sem_poison
    self.nc.clear_and_free_semaphores(list(self.sems.allocated().values()))
    self.nc.all_engine_barrier()


tile.TileContext._drain_and_barrier = _patched_drain_and_barrier


def _dedup_ldweights(nc):
    """Tile lowering splits every matmul into Ldweights+Matmult; with walrus
    ldw-opt disabled each pair reloads the stationary operand even when
    consecutive matmuls share it. Replace redundant Ldweights (same weights
    AP + tile params, tracked PER tile_position, only Matmults in between on
    PE) with NoOps that keep their sync_info."""
    n_removed = 0
    for f in nc.m.functions:
        for bb in f.blocks:
            last_key = {}
            insts = bb.instructions
            for idx, inst in enumerate(insts):
                tname = type(inst).__name__
                if tname == "InstLdweights":
                    pos = str(getattr(inst, "tile_position", None))
                    key = (
                        str(inst.ins[0]),
                        str(getattr(inst, "tile_size", None)),
                        str(getattr(inst, "perf_mode", None)),
                        str(getattr(inst, "is_transpose", None)),
                    )
                    if last_key.get(pos) == key:
                        nop = mybir.InstNoOp(
                            name=f"I-ldwdedup-{n_removed}", ins=[], outs=[])
                        nop.engine = inst.engine
                        nop.sync_info = inst.sync_info
                        insts[idx] = nop
                        n_removed += 1
                    else:
                        last_key[pos] = key
                elif tname == "InstMatmult" or inst.engine != mybir.EngineType.PE:
                    continue
                else:
                    last_key = {}
    return n_removed


def _split_multiwait_instructions(nc):
    """Same walrus limitation, applied generically: any instruction whose
    sync header carries >1 wait gets its extra waits moved onto NoOps
    inserted just before it on the same engine (per-engine order is the
    block-list order filtered by engine, so this preserves semantics)."""
    ctr = [0]
    for f in nc.m.functions:
        for bb in f.blocks:
            insts = bb.instructions
            out = []
            for inst in insts:
                si = getattr(inst, "sync_info", None)
                waits = list(si.on_wait) if si is not None and si.on_wait else []
                if len(waits) > 1:
                    for w in waits[:-1]:
                        nop = mybir.InstNoOp(
                            name=f"I-waitsplit-{ctr[0]}", ins=[], outs=[])
                        ctr[0] += 1
                        nop.engine = inst.engine
                        nop.sync_info = mybir.SyncInfo(
                            on_wait=[w], on_update=[])
                        out.append(nop)
                    inst.sync_info = mybir.SyncInfo(
                        on_wait=[waits[-1]],
                        on_update=list(si.on_update or []))
                out.append(inst)
            if len(out) != len(insts):
                insts[:] = out

CIN = 2048
C = 512
H = 64
W = 64
KT = CIN // 128   # 16 cin tiles
MT = C // 128     # 4 cout tiles
ROWS = 34         # row 0 = edge pad (zeros from host), 1..32 owned, 33 = halo
WPAD = 68         # xr pad layout: data cols 2..65; taps read cols 1..66

ROW_BLOCKS = [(1, 9), (9, 17), (17, 25), (25, 33)]  # xr rows (halo deferred)
OUT_BLOCKS = [(0, 8), (8, 16), (16, 24), (24, 32)]            # output rows
DVE_DW = [(0, 3), (1, 3), (2, 3), (3, 3)]  # depthwise units offloaded to DVE

_CACHE = {}


def _l1_bins_view(t):
    """[128, 3(q), 11(rows 22..32), 22(w)] overlapping-bin view of an
    [128, 34, 64] tile: w-bin starts {0, 21, 42} (step 21)."""
    import bass_rust
    v = t[:].copy()
    v.ap = bass_rust.VecI64Pair([[34 * 64, 128], [21, 3], [64, 11], [1, 22]])
    v.offset = 22 * 64
    return v


def _l0q2_view(t):
    """[128, 23(rows 0..22), 22(w 42..63)] view for the L0 q=2 w-bin."""
    import bass_rust
    v = t[:].copy()
    v.ap = bass_rust.VecI64Pair([[34 * 64, 128], [64, 23], [1, 22]])
    v.offset = 42
    return v


def build_graph():
    nc = bass.Bass(num_devices=8)

    x_in = nc.declare_dram_parameter("x_sh", [CIN, ROWS, W], BF16,
                                     isOutput=False)
    # weights pre-transposed host-side to partition-major [128, k, c] so the
    # DMA access pattern is contiguous per partition
    reduT_d = nc.declare_dram_parameter("reduT", [128, KT, C], BF16,
                                        isOutput=False)
    fgwT_d = nc.declare_dram_parameter("fgwT", [128, KT, C], BF16,
                                       isOutput=False)
    fuT_d = nc.declare_dram_parameter("fuT", [128, MT, C], BF16,
                                      isOutput=False)
    # blob layout: rb[0:4] gb[4:8] fb[8:12] maskgf[12:30] mask9[30:174]
    blob_d = nc.declare_dram_parameter("blob", [128, 174], F32, isOutput=False)
    eye_d = nc.declare_dram_parameter("eye", [128, 128], BF16, isOutput=False)
    out_d = nc.declare_dram_parameter("out", [C, 32, W], BF16, isOutput=True)

    # pool exchange buffers (bf16: halves the collective payload)
    pool_part = nc.dram_tensor("pool_part", [128, KT * 9], BF16)
    pool_red = nc.dram_tensor("pool_red", [128, KT * 9], BF16)
    warm_in_d = nc.dram_tensor("warm_in", [128, 2], F32)
    warm_d = nc.dram_tensor("warm", [128, 2], F32)
    wake1_d = nc.dram_tensor("wake1", [128, 8], F32)
    wake2_d = nc.dram_tensor("wake2", [128, 8], BF16)
    wake3_d = nc.dram_tensor("wake3", [128, 8], BF16)

    AF = mybir.ActivationFunctionType
    OP = mybir.AluOpType
    GROUPS = [[0, 1], [2, 3], [4, 5], [6, 7]]

    with tile.TileContext(nc) as tc:
        with (
            tc.tile_pool(name="const", bufs=1) as const,
            tc.tile_pool(name="work", bufs=2) as work,
            tc.tile_pool(name="dw", bufs=1) as dwp,
            tc.tile_pool(name="osb", bufs=6) as osbp,
            tc.tile_pool(name="ps", bufs=8, space="PSUM") as ps,
        ):
            # ---- warm-up AllReduce on an uninitialized scratch (the summed
            # garbage is discarded): with no producer dependency it triggers
            # as soon as GpSimd boots (~7.5us), burning the ~43us ncfw
            # first-op latency (counted from the FIRST trigger) early ----
            nc.gpsimd.collective_compute(
                "AllReduce", OP.add, replica_groups=GROUPS,
                ins=[warm_in_d[:, :]], outs=[warm_d[:, :]])

            # ---- sync ring: the whole x stream (splitting x across rings
            # only redistributes the fair-share bandwidth, measured slower) ----
            xbf = []
            for k in range(KT):
                xbf.append(const.tile([128, ROWS, W], BF16, tag=f"xbf{k}",
                                      name=f"xbf{k}"))
            for k in range(KT):
                nc.sync.dma_start(xbf[k][:], x_in[k * 128:(k + 1) * 128, :, :])

            # ---- scalar ring: reduT k0 slice first (first matmul dep),
            # then the rest + blob + eye; fgwT/fuT deferred past the x
            # stream via an ACT gate op below ----
            reduT0_sb = const.tile([128, 1, C], BF16, tag="reduT0")
            nc.scalar.dma_start(reduT0_sb[:], reduT_d[:, 0:1, :])
            reduT1_sb = const.tile([128, 5, C], BF16, tag="reduT1")
            nc.scalar.dma_start(reduT1_sb[:], reduT_d[:, 1:6, :])
            blob_sb = const.tile([128, 174], F32, tag="blob")
            nc.scalar.dma_start(blob_sb[:], blob_d[:])
            eye_sb = const.tile([128, 128], BF16, tag="eye")
            nc.scalar.dma_start(eye_sb[:], eye_d[:])
            # reduT k6..15 is gated behind xbf5 (inside the k-loop below) so
            # the scalar ring stays quiet through the slow DMA ramp
            reduT2_sb = const.tile([128, KT - 6, C], BF16, tag="reduT2")
            fgwT_sb = const.tile([128, KT, C], BF16, tag="fgwT")
            fuT_sb = const.tile([128, MT, C], BF16, tag="fuT")

            rb_sb = blob_sb[:, 0:4]
            gb_sb = blob_sb[:, 4:8]
            fb_sb = blob_sb[:, 8:12]
            maskgf_sb = blob_sb[:, 12:30]
            mask9_sb = blob_sb[:, 30:174].rearrange("p (k q) -> p k q", q=9)

            def reduT_w(k, m):
                if k < 1:
                    return reduT0_sb[:, 0, m * 128:(m + 1) * 128]
                if k < 6:
                    return reduT1_sb[:, k - 1, m * 128:(m + 1) * 128]
                return reduT2_sb[:, k - 6, m * 128:(m + 1) * 128]

            # ---- xr targets (pad rows/cols zeroed once) ----
            xr = []
            for m in range(MT):
                t = const.tile([128, ROWS, WPAD], BF16, tag=f"xr{m}",
                               name=f"xr{m}")
                xr.append(t)
                nc.gpsimd.memset(t[:, 0:1, :], 0.0)        # edge pad row
                nc.gpsimd.memset(t[:, :, 1:2], 0.0)        # left pad col
                nc.gpsimd.memset(t[:, :, 66:67], 0.0)      # right pad col

            # ---- pass A (m0,m1, no halo) streamed with x; pool partials
            # per arriving tile: L0 q0/q1 on ACT (accum_out), L0 q2 + all
            # of L1 on DVE — splits the read load so both keep pace ----
            pool_acc = work.tile([128, KT, 6], F32, tag="pacc", bufs=1)
            dup = work.tile([128, KT, 9], F32, tag="dup", bufs=1)
            scat = work.tile([128, KT, 9], BF16, tag="scat", bufs=1)
            pooled_bf = work.tile([128, KT, 9], BF16, tag="poolbf", bufs=1)
            psA = {m: [ps.tile([128, 8, W], F32, tag="ps", name=f"psr{m}_{bi}")
                       for bi in range(len(ROW_BLOCKS))] for m in (0, 1)}
            for k in range(KT):
                for q in range(2):
                    pdump = work.tile([128, 23, 22], BF16, tag="pdump",
                                      name="pdump")
                    nc.scalar.activation(
                        out=pdump[:, 0:23, :],
                        in_=xbf[k][:, 0:23, 21 * q:21 * q + 22],
                        func=AF.Copy,
                        accum_out=pool_acc[:, k, q:q + 1],
                    )
                nc.vector.tensor_reduce(
                    out=pool_acc[:, k, 2:3],
                    in_=_l0q2_view(xbf[k]),
                    axis=mybir.AxisListType.XY,
                    op=OP.add,
                )
                nc.vector.tensor_reduce(
                    out=pool_acc[:, k, 3:6],
                    in_=_l1_bins_view(xbf[k]),
                    axis=mybir.AxisListType.XY,
                    op=OP.add,
                )
                if k == 5:
                    # gate the bulk reduT pull until the DMA ramp is over
                    # (k6 is consumed ~3us after xbf5 lands; the ring
                    # delivers each 128KB slice in ~0.5us)
                    gate5 = work.tile([128, 1], F32, tag="gate5", bufs=1)
                    nc.scalar.activation(out=gate5[:], in_=xbf[5][:, 0, 0:1],
                                         func=AF.Copy)
                    nc.scalar.dma_start(reduT2_sb[:], reduT_d[:, 6:16, :])
                if k == 13:
                    # ACT gate: delay fgwT/fuT ring traffic until the x
                    # stream is nearly done (needed only ~30us later)
                    gate = work.tile([128, 1], F32, tag="gate", bufs=1)
                    nc.scalar.activation(out=gate[:], in_=xbf[13][:, 0, 0:1],
                                         func=AF.Copy)
                    nc.scalar.dma_start(fgwT_sb[:], fgwT_d[:])
                    nc.scalar.dma_start(fuT_sb[:], fuT_d[:])
                if k == KT - 1:
                    # scatter + dump + AllReduce, all GpSimd-local (in-order
                    # on an idle engine; SWDGE re-wakes in ~1.5us, unlike
                    # the HWDGE rings' ~9us)
                    with tc.high_priority():
                        nc.gpsimd.tensor_copy(dup[:, :, 0:6],
                                              pool_acc[:, :, 0:6])
                        nc.gpsimd.tensor_copy(dup[:, :, 6:9],
                                              pool_acc[:, :, 0:3])
                        nc.gpsimd.tensor_mul(scat[:], dup[:], mask9_sb[:])
                        nc.gpsimd.dma_start(pool_part[:, :], scat[:].rearrange(
                            "p k q -> p (k q)"))
                        nc.gpsimd.collective_compute(
                            "AllReduce", OP.add, replica_groups=GROUPS,
                            ins=[pool_part[:, :]], outs=[pool_red[:, :]])
                        nc.gpsimd.dma_start(
                            pooled_bf[:].rearrange("p k q -> p (k q)"),
                            pool_red[:, :])
                for m in (0, 1):
                    for bi, (r0, r1) in enumerate(ROW_BLOCKS):
                        nc.tensor.matmul(
                            psA[m][bi][:],
                            reduT_w(k, m),
                            xbf[k][:, r0:r1, :],
                            start=(k == 0), stop=(k == KT - 1),
                        )

            # ---- evictions: xr rows = relu(psum + redu bias), all on ACT
            # (DVE is saturated by pool stage-1 during the stream) ----
            def evict(m, r0, r1, src, on_dve=False):
                if on_dve:
                    nc.vector.tensor_scalar(
                        out=xr[m][:, r0:r1, 2:66], in0=src,
                        scalar1=rb_sb[:, m:m + 1], scalar2=0.0,
                        op0=OP.add, op1=OP.max)
                else:
                    nc.scalar.activation(
                        out=xr[m][:, r0:r1, 2:66], in_=src,
                        func=AF.Relu, bias=rb_sb[:, m:m + 1])

            for i, (m, bi) in enumerate(
                    [(m, bi) for m in (0, 1) for bi in range(4)]):
                r0, r1 = ROW_BLOCKS[bi]
                evict(m, r0, r1, psA[m][bi][:], on_dve=(i % 2 == 0))

            # ---- pass B: m2 then m3, each k-loop also carrying two halo
            # rows (own m + one pass-A m) so the halo ldweights overlap the
            # 512-col block matmuls instead of forming a standalone
            # ldw-bound phase (~9us for 1.7us of useful columns) ----
            for m, mh in ((2, 0), (3, 1)):
                pst = [ps.tile([128, 8, W], F32, tag="ps", name=f"psr{m}_{bi}")
                       for bi in range(len(ROW_BLOCKS))]
                ph_own = ps.tile([128, 1, W], F32, tag="ps", name=f"psh{m}")
                ph_oth = ps.tile([128, 1, W], F32, tag="ps", name=f"psh{mh}")
                for k in range(KT):
                    for bi, (r0, r1) in enumerate(ROW_BLOCKS):
                        nc.tensor.matmul(
                            pst[bi][:], reduT_w(k, m), xbf[k][:, r0:r1, :],
                            start=(k == 0), stop=(k == KT - 1),
                        )
                    nc.tensor.matmul(
                        ph_own[:], reduT_w(k, m), xbf[k][:, 33:34, :],
                        start=(k == 0), stop=(k == KT - 1),
                    )
                    nc.tensor.matmul(
                        ph_oth[:], reduT_w(k, mh), xbf[k][:, 33:34, :],
                        start=(k == 0), stop=(k == KT - 1),
                    )
                for bi, (r0, r1) in enumerate(ROW_BLOCKS):
                    evict(m, r0, r1, pst[bi][:], on_dve=(bi % 2 == 0))
                evict(m, 33, 34, ph_own[:], on_dve=(m % 2 == 0))
                evict(mh, 33, 34, ph_oth[:], on_dve=(mh % 2 == 1))

            # ---- filter-gen matmul (pooled lands ~10us before m3 ends);
            # per-m taps (DVE, per-core mirror via host masks) emitted right
            # after each gen tile so the diag builds overlap later gen m ----
            gen_acc = work.tile([128, 36], F32, tag="genacc", bufs=1)
            gfu = [None] * MT
            for m in range(MT):
                pg = ps.tile([128, 16], F32, tag="ps", name=f"psg{m}")
                for k in range(KT):
                    nc.tensor.matmul(
                        pg[:, 0:9],
                        fgwT_sb[:, k, m * 128:(m + 1) * 128],
                        pooled_bf[:, k, :],
                        start=(k == 0), stop=(k == KT - 1),
                    )
                nc.vector.tensor_copy(gen_acc[:, m * 9:(m + 1) * 9], pg[:, 0:9])
                gf = work.tile([128, 9], F32, tag="gf")
                nc.vector.tensor_scalar_add(
                    gf[:], gen_acc[:, m * 9:(m + 1) * 9], gb_sb[:, m:m + 1])
                gfdup = work.tile([128, 18], F32, tag="gfdup")
                nc.vector.tensor_copy(gfdup[:, 0:9], gf[:])
                for dy in range(3):
                    nc.vector.tensor_copy(
                        gfdup[:, 9 + 3 * dy:12 + 3 * dy],
                        gf[:, 3 * (2 - dy):3 * (2 - dy) + 3])
                gft = work.tile([128, 18], F32, tag="gft")
                nc.vector.tensor_mul(gft[:], gfdup[:], maskgf_sb[:])
                g = const.tile([128, 9], F32, tag=f"gfu{m}", name=f"gfu{m}")
                nc.vector.tensor_add(g[:], gft[:, 0:9], gft[:, 9:18])
                gfu[m] = g

            # sync-ring pre-wake #1 (fires with gen_acc, ~16us before the
            # first output chunk needs the ring)
            nc.sync.dma_start(wake1_d[:, :], gen_acc[:, 0:8])

            # ---- diag tiles on ACT (activation scale = per-channel tap),
            # t-major so PE's tap loop never waits ----
            diag = [[None] * 9 for _ in range(MT)]
            for t in range(9):
                for m in range(MT):
                    d = const.tile([128, 128], BF16, tag=f"dg{m}_{t}",
                                   name=f"dg{m}_{t}")
                    nc.scalar.activation(
                        out=d[:], in_=eye_sb[:], func=AF.Copy,
                        scale=gfu[m][:, t:t + 1])
                    diag[m][t] = d

            # sync-ring pre-wake #2 (fires when the last diags build)
            nc.sync.dma_start(wake2_d[:, :], diag[0][8][:, 0:8])

            # ---- DVE depthwise units (bi=3) via scalar_tensor_tensor on
            # FLAT 68-wide row windows: contiguous reads run ~20% faster
            # than the strided 64-of-68 views; the 4 pad lanes per row
            # accumulate junk that the final strided relu never reads, and
            # windows are clipped to the xr tile end ----
            dw_bf = [[None] * len(OUT_BLOCKS) for _ in range(MT)]
            NF = 8 * WPAD  # 544
            for (m, bi) in DVE_DW:
                o0, o1 = OUT_BLOCKS[bi]
                xf = xr[m][:].rearrange("p r c -> p (r c)")
                pa = work.tile([128, NF], F32, tag="dva", bufs=2)
                pb = work.tile([128, NF], F32, tag="dvb", bufs=2)
                base = o0 * WPAD + 1
                nc.vector.tensor_scalar_mul(
                    pa[:], xf[:, base:base + NF], gfu[m][:, 0:1])
                cur, nxt = pa, pb
                for t in range(1, 9):
                    dy, dx = t // 3, t % 3
                    b = (o0 + dy) * WPAD + dx + 1
                    L = min(NF, ROWS * WPAD - b)
                    nc.vector.scalar_tensor_tensor(
                        out=nxt[:, 0:L],
                        in0=xf[:, b:b + L],
                        scalar=gfu[m][:, t:t + 1], in1=cur[:, 0:L],
                        op0=OP.mult, op1=OP.add)
                    cur, nxt = nxt, cur
                dd = dwp.tile([128, 8, W], BF16, tag=f"dwbf{m}_{bi}",
                              name=f"dwbf{m}_{bi}")
                nc.vector.tensor_scalar_max(
                    dd[:], cur[:].rearrange("p (r c) -> p r c", c=WPAD)[
                        :, :, 0:64], 0.0)
                dw_bf[m][bi] = dd

            # ---- PE depthwise + fusion, interleaved by row-block pair ----
            PAIRS = [((0, 1), None), ((2, 3), DVE_DW)]
            first_wake3 = [True]
            for bis, skip in PAIRS:
                skip = skip or []
                units = [(m, bi) for m in range(MT) for bi in bis
                         if (m, bi) not in skip]
                pdm = {}
                for (m, bi) in units:
                    pdm[(m, bi)] = ps.tile([128, 8, W], F32, tag="ps",
                                           name=f"psd{m}_{bi}")
                for t in range(9):
                    dy, dx = t // 3, t % 3
                    for (m, bi) in units:
                        o0, o1 = OUT_BLOCKS[bi]
                        nc.tensor.matmul(
                            pdm[(m, bi)][:],
                            diag[m][t][:, :],
                            xr[m][:, o0 + dy:o1 + dy, dx + 1:dx + 65],
                            start=(t == 0), stop=(t == 8),
                        )
                # PE-unit evictions stay off DVE: its queue holds the long
                # scalar_tensor_tensor unit chains, which would delay these
                for (m, bi) in units:
                    d = dwp.tile([128, 8, W], BF16, tag=f"dwbf{m}_{bi}",
                                 name=f"dwbf{m}_{bi}")
                    nc.scalar.activation(out=d[:], in_=pdm[(m, bi)][:],
                                         func=AF.Relu)
                    dw_bf[m][bi] = d
                if first_wake3[0]:
                    # pre-wake #3: fires with the first dw eviction
                    nc.sync.dma_start(wake3_d[:, :],
                                      dw_bf[0][bis[0]][:, 0, 0:8])
                    first_wake3[0] = False
                # fusion co-major: each co's PSUM stops ~2us apart, so the
                # evict + output DMA pipeline drains during the matmuls
                # instead of all at the end. fus01 evicts: ACT only (DVE is
                # busy with unit chains); fus23: split ACT/DVE.
                for co in range(MT):
                    pfc = {bi: ps.tile([128, 8, W], F32, tag="ps",
                                       name=f"psf{co}_{bi}") for bi in bis}
                    for kc in range(MT):
                        for bi in bis:
                            nc.tensor.matmul(
                                pfc[bi][:],
                                fuT_sb[:, kc, co * 128:(co + 1) * 128],
                                dw_bf[kc][bi][:],
                                start=(kc == 0), stop=(kc == MT - 1),
                            )
                    for i, bi in enumerate(bis):
                        o0, o1 = OUT_BLOCKS[bi]
                        ch = osbp.tile([128, 8, W], BF16, tag="osb",
                                       name="osb")
                        if bis[0] == 0 or (co * 2 + i) % 2 == 0:
                            nc.scalar.activation(
                                out=ch[:], in_=pfc[bi][:], func=AF.Relu,
                                bias=fb_sb[:, co:co + 1])
                        else:
                            nc.vector.tensor_scalar(
                                out=ch[:], in0=pfc[bi][:],
                                scalar1=fb_sb[:, co:co + 1], scalar2=0.0,
                                op0=OP.add, op1=OP.max)
                        nc.sync.dma_start(
                            out_d[co * 128:(co + 1) * 128, o0:o1, :], ch[:])
    _dedup_ldweights(nc)
    _split_multiwait_instructions(nc)
    return nc


def _host_inputs(x, filter_gen_w, filter_gen_b, redu_w, redu_b, fusion_w,
                 fusion_b):
    bf = ml_dtypes.bfloat16

    def pmajor(wT, kt):
        # [Cin, C] -> [128, kt, C]: partition-major so the DMA is contiguous
        return np.ascontiguousarray(
            wT.reshape(kt, 128, -1).transpose(1, 0, 2)).astype(bf)

    x = x.astype(bf)
    shared = {
        "reduT": pmajor(redu_w.T, KT),
        "fgwT": pmajor((filter_gen_w / 484.0).T, KT),
        "fuT": pmajor(fusion_w.T, MT),
        "eye": np.eye(128, dtype=bf),
    }
    rb4 = np.ascontiguousarray(redu_b.reshape(MT, 128).T)
    gb4 = np.ascontiguousarray(filter_gen_b.reshape(MT, 128).T)
    fb4 = np.ascontiguousarray(fusion_b.reshape(MT, 128).T)
    in_maps = []
    for i in range(8):
        b, half = i // 2, i % 2
        if half == 0:
            rows = x[b, :, 0:33, :]
            m9 = [1, 1, 1, 1, 1, 1, 0, 0, 0]
            mgf = [1.0] * 9 + [0.0] * 9
        else:
            rows = x[b, :, 63:30:-1, :]
            m9 = [0, 0, 0, 1, 1, 1, 1, 1, 1]
            mgf = [0.0] * 9 + [1.0] * 9
        xs = np.concatenate(
            [np.zeros((CIN, 1, W), bf), rows], axis=1)
        blob = np.concatenate([
            rb4, gb4, fb4,
            np.tile(np.asarray(mgf, np.float32), (128, 1)),
            np.tile(np.asarray(m9, np.float32), (128, KT)),
        ], axis=1)
        assert blob.shape == (128, 174), blob.shape
        in_maps.append({
            **shared,
            "x_sh": np.ascontiguousarray(xs),
            "blob": np.ascontiguousarray(blob),
        })
    return in_maps


def kernel(x, filter_gen_w, filter_gen_b, redu_w, redu_b, fusion_w, fusion_b):
    x = np.asarray(x, np.float32)
    if "nc" not in _CACHE:
        _CACHE["nc"] = build_graph()
    nc = _CACHE["nc"]
    in_maps = _host_inputs(
        x, np.asarray(filter_gen_w, np.float32),
        np.asarray(filter_gen_b, np.float32),
        np.asarray(redu_w, np.float32), np.asarray(redu_b, np.float32),
        np.asarray(fusion_w, np.float32), np.asarray(fusion_b, np.float32))
    trace = os.environ.get("KERNEL_TRACE") == "1"
    res = run_bass_kernel_spmd(nc, in_maps, list(range(8)), trace=trace)
    if res.exec_time_ns is not None:
        print(f"HW exec time: {res.exec_time_ns} ns")
    out = np.zeros((4, C, H, W), np.float32)
    for i in range(8):
        b, half = i // 2, i % 2
        r = np.asarray(res.results[i]["out"]).astype(np.float32)
        if half == 0:
            out[b, :, 0:32] = r
        else:
            out[b, :, 32:64] = r[:, ::-1, :]
    return out
